# revision 24
# baseline (speedup 1.0000x reference)
"""Trainium2 Bass kernel: RMSNorm + RoPE + causal attention + output projection.

Tensor-parallel over heads: 16 heads / 8 cores = 2 heads per core.
Each core computes a full [S, D] partial output; the all-reduce is done
host-side in the gather.

v3 design (all-bf16 matmuls, host-transposed activations, fused single pass):
  - xs is transposed on the host: x^T [D, S] bf16 is DMA'd straight into
    SBUF per chunk — no PE transposes of activations, no PSUM->SBUF copies
    for h^T at all.
  - RMSNorm: sum-of-squares via a PE gram accumulation on the x^T tiles
    (diagonal extracted with one fused DVE op per s-tile), and the istd
    scale folded into per-chunk scaled rope tables (Q,K) and an istd
    multiply on V^T. h^T itself stays unscaled.
  - istd broadcast [s] -> [128, s-chunk] via a tiny fp32r transpose plus
    4 selector matmuls (proven in probe).
  - All PE matmuls bf16 (FWL fast weight loads), fp32 PSUM accumulation.
  - Attention per head with paired key-tiles: exp batched [128,1024] on
    the scalar engine, Z via ones-matmul, PV accumulation, LAG-2 software
    pipeline; causal handled by full-width diagonal scores + triangular
    mask + column-offset slicing of Z/PV.
  - Output projection inlined per chunk, drains split scalar/DVE.
  - PE warmup matmuls on junk data cover the initial DMA window.
"""
import os
import sys
import types

import numpy as np
import ml_dtypes

SEQ, D, NH, HD = 4096, 2048, 16, 128
NCORES = 8
HPC = NH // NCORES          # heads per core
M = HPC * HD                # per-core fused head dim (256)
EPS = 1e-6
ROPE_BASE = 10000.0
SM_SCALE = 1.0 / np.sqrt(HD)
CHUNK = 512                 # q-chunk
NCHUNK = SEQ // CHUNK       # 8
DT = D // 128               # 16 d-tiles
NWARM = 44                  # PE warmup matmuls during initial DMA window


def _inject_ntff_hook():
    """Register the axon NTFF profiling hook (missing antenv.axon_hooks)."""
    if "antenv.axon_hooks" in sys.modules:
        return
    try:
        import antenv
        from trn_agent_boot.trn_boot import _ntff_profile_via_ctypes
    except ImportError:
        return
    holder = [None]
    mod = types.ModuleType("antenv.axon_hooks")
    mod.set_axon_ntff_profile_hook = lambda h: holder.__setitem__(0, h)
    mod.get_axon_ntff_profile_hook = lambda: holder[0]
    sys.modules["antenv.axon_hooks"] = mod
    antenv.axon_hooks = mod
    try:
        mod.set_axon_ntff_profile_hook(
            _ntff_profile_via_ctypes("/opt/axon/libaxon_pjrt.so"))
    except Exception:
        pass


def _build_nc():
    import concourse.bass as bass  # noqa: F401
    import concourse.mybir as mybir
    import concourse.tile as tile
    from concourse import bacc

    FP32 = mybir.dt.float32
    FP32R = mybir.dt.float32r
    BF16 = mybir.dt.bfloat16
    U32 = mybir.dt.uint32
    AF = mybir.ActivationFunctionType
    ALU = mybir.AluOpType

    nc = bacc.Bacc(None, target_bir_lowering=False)

    xsT = nc.declare_dram_parameter("xsT", [D, SEQ], BF16, isOutput=False)
    wq = nc.declare_dram_parameter("wq", [D, M], BF16, isOutput=False)
    wk = nc.declare_dram_parameter("wk", [D, M], BF16, isOutput=False)
    wv = nc.declare_dram_parameter("wv", [D, M], BF16, isOutput=False)
    wo = nc.declare_dram_parameter("wo", [D, M], BF16, isOutput=False)
    cosd = nc.declare_dram_parameter("cosd", [128, SEQ], BF16, isOutput=False)
    sind = nc.declare_dram_parameter("sind", [128, SEQ], BF16, isOutput=False)
    tri = nc.declare_dram_parameter("tri", [128, 128], BF16, isOutput=False)
    ones = nc.declare_dram_parameter("ones", [128, 128], BF16, isOutput=False)
    ident = nc.declare_dram_parameter("ident", [128, 128], BF16, isOutput=False)
    identr = nc.declare_dram_parameter("identr", [128, 128], FP32R,
                                       isOutput=False)
    sel4 = nc.declare_dram_parameter("sel4", [4, 512], FP32R, isOutput=False)
    out = nc.declare_dram_parameter("out", [SEQ, D], FP32, isOutput=True)

    with tile.TileContext(nc) as tc:
        with tc.tile_pool(name="consts", bufs=1) as consts, \
             tc.tile_pool(name="state", bufs=1) as state, \
             tc.tile_pool(name="ht", bufs=2) as htp, \
             tc.tile_pool(name="nrm", bufs=2) as nrm, \
             tc.tile_pool(name="sct", bufs=2) as sct, \
             tc.tile_pool(name="qtc", bufs=2) as qtcp, \
             tc.tile_pool(name="vst", bufs=2) as vstp, \
             tc.tile_pool(name="rp", bufs=2) as rpp, \
             tc.tile_pool(name="pt", bufs=5) as ptp, \
             tc.tile_pool(name="rz", bufs=2) as rzp, \
             tc.tile_pool(name="att", bufs=2) as attp, \
             tc.tile_pool(name="ost", bufs=2) as ostp, \
             tc.tile_pool(name="big", bufs=2, space="PSUM") as big, \
             tc.tile_pool(name="oz", bufs=2, space="PSUM") as ozp:

            # ---- warmup: keep PE busy during the initial DMA window ----
            junk = consts.tile([128, 512], BF16)
            nc.vector.memset(junk[:], 0.125)
            for _ in range(NWARM):
                wm = big.tile([128, 512], FP32, name="warm", tag="big")
                nc.tensor.matmul(wm[:], junk[:, 0:128], junk[:],
                                 start=True, stop=True)

            # ---- setup DMAs (consts on the vector queue; activations and
            # weights on sync/scalar queues so they don't serialize) ----
            ident_sb = consts.tile([128, 128], BF16)
            nc.scalar.dma_start(out=ident_sb[:], in_=ident[:])
            idr_sb = consts.tile([128, 128], FP32R)
            nc.scalar.dma_start(out=idr_sb[:], in_=identr[:])
            sel_sb = consts.tile([4, 512], FP32R)
            nc.scalar.dma_start(out=sel_sb[:], in_=sel4[:])
            magic_sb = consts.tile([128, 4], U32)
            nc.vector.memset(magic_sb[:], 0x5F3759DF)

            # persistent per-head state
            kt = [state.tile([128, SEQ], BF16, name=f"kt{h}") for h in range(HPC)]
            vn = [state.tile([128, SEQ], BF16, name=f"vn{h}") for h in range(HPC)]
            wot = [state.tile([128, D], BF16, name=f"wot{h}") for h in range(HPC)]

            def emit_ht_dma(c):
                ht = htp.tile([128, DT * CHUNK], BF16, name="ht")
                nc.sync.dma_start(
                    out=ht[:].rearrange("p (t s) -> p t s", t=DT),
                    in_=xsT[:, c * CHUNK:(c + 1) * CHUNK].rearrange(
                        "(t p) s -> p t s", p=128))
                return ht

            # chunk 0 activations first, then weights/tables
            ht_cur = emit_ht_dma(0)

            wq_sb = consts.tile([128, DT * M], BF16)
            nc.sync.dma_start(out=wq_sb[:].rearrange("p (t m) -> p t m", t=DT),
                              in_=wq[:].rearrange("(t p) m -> p t m", p=128))
            wk_sb = consts.tile([128, DT * M], BF16)
            nc.scalar.dma_start(out=wk_sb[:].rearrange("p (t m) -> p t m", t=DT),
                                in_=wk[:].rearrange("(t p) m -> p t m", p=128))
            wv_sb = consts.tile([128, DT * M], BF16)
            nc.scalar.dma_start(out=wv_sb[:].rearrange("p (t m) -> p t m", t=DT),
                                in_=wv[:].rearrange("(t p) m -> p t m", p=128))
            cos_sb = consts.tile([128, SEQ], BF16)
            nc.scalar.dma_start(out=cos_sb[:], in_=cosd[:])
            sin_sb = consts.tile([128, SEQ], BF16)
            nc.scalar.dma_start(out=sin_sb[:], in_=sind[:])
            tri_sb = consts.tile([128, 128], BF16)
            nc.scalar.dma_start(out=tri_sb[:], in_=tri[:])
            ones_sb = consts.tile([128, 128], BF16)
            nc.scalar.dma_start(out=ones_sb[:], in_=ones[:])
            wo_sb = consts.tile([128, DT * M], BF16)
            nc.scalar.dma_start(out=wo_sb[:].rearrange("p (t m) -> p t m", t=DT),
                                in_=wo[:].rearrange("(t p) m -> p t m", p=128))

            # ---- norm stats, DVE part: gram diag -> istd4 (bit-hack rsqrt,
            # no scalar-engine op => no activation-table thrash) ----
            def emit_stats_gram(c, ht):
                gram = ozp.tile([128, 512], FP32, name="gram", tag="oz")
                for st in range(4):
                    for dt in range(DT):
                        blk = ht[:, dt * CHUNK + st * 128:
                                 dt * CHUNK + (st + 1) * 128]
                        nc.tensor.matmul(gram[:, st * 128:(st + 1) * 128],
                                         blk, blk,
                                         start=(dt == 0), stop=(dt == DT - 1))
                scr = nrm.tile([128, 128], BF16, name="scr")
                ssq4 = nrm.tile([128, 4], FP32, name="ssq4")
                for st in range(4):
                    nc.vector.scalar_tensor_tensor(
                        out=scr[:], in0=gram[:, st * 128:(st + 1) * 128],
                        scalar=1.0, in1=ident_sb[:],
                        op0=ALU.mult, op1=ALU.mult,
                        accum_out=ssq4[:, st:st + 1])
                # istd = rsqrt(ssq/D + eps): bit-hack + 2 Newton iterations
                ms = nrm.tile([128, 4], FP32, name="ms")
                nc.vector.tensor_scalar(out=ms[:], in0=ssq4[:],
                                        scalar1=1.0 / D, scalar2=EPS,
                                        op0=ALU.mult, op1=ALU.add)
                ih = nrm.tile([128, 4], U32, name="ih")
                nc.vector.tensor_scalar(out=ih[:], in0=ms[:].bitcast(U32),
                                        scalar1=1, scalar2=None,
                                        op0=ALU.logical_shift_right)
                y = nrm.tile([128, 4], FP32, name="y")
                nc.vector.scalar_tensor_tensor(
                    out=y[:].bitcast(U32), in0=magic_sb[:], scalar=0,
                    in1=ih[:], op0=ALU.bypass, op1=ALU.subtract)
                t = nrm.tile([128, 4], FP32, name="t")
                istd4 = nrm.tile([128, 4], FP32R, name="istd4")
                for it in range(2):
                    nc.vector.tensor_tensor(out=t[:], in0=y[:], in1=y[:],
                                            op=ALU.mult)
                    nc.vector.tensor_tensor(out=t[:], in0=t[:], in1=ms[:],
                                            op=ALU.mult)
                    nc.vector.tensor_scalar(out=t[:], in0=t[:], scalar1=-0.5,
                                            scalar2=1.5, op0=ALU.mult,
                                            op1=ALU.add)
                    dst = y[:] if it == 0 else istd4[:]
                    with nc.allow_low_precision(reason="istd bcast chain"):
                        nc.vector.tensor_tensor(out=dst, in0=y[:], in1=t[:],
                                                op=ALU.mult)
                return istd4

            # ---- norm stats, PE part: broadcast istd + scaled tables ----
            def emit_stats_bcast(c, istd4):
                csl = slice(c * CHUNK, (c + 1) * CHUNK)
                itT = big.tile([4, 128], FP32R, name="itT", tag="big")
                nc.tensor.transpose(itT[:], istd4[:], idr_sb[:])
                itT_sb = nrm.tile([4, 128], FP32R, name="itT_sb")
                nc.vector.tensor_copy(itT_sb[:], itT[:])
                bc = big.tile([128, 512], FP32, name="bc", tag="big")
                for st in range(4):
                    nc.tensor.matmul(bc[:, st * 128:(st + 1) * 128],
                                     sel_sb[:, st * 128:(st + 1) * 128],
                                     itT_sb[:], start=True, stop=True)
                cos_sc = sct.tile([128, CHUNK], BF16, name="cos_sc")
                nc.vector.tensor_tensor(out=cos_sc[:], in0=bc[:],
                                        in1=cos_sb[:, csl], op=ALU.mult)
                sin_sc = sct.tile([128, CHUNK], BF16, name="sin_sc")
                nc.vector.tensor_tensor(out=sin_sc[:], in0=bc[:],
                                        in1=sin_sb[:, csl], op=ALU.mult)
                istd_bc = sct.tile([128, CHUNK], FP32, name="istd_bc")
                nc.vector.tensor_copy(istd_bc[:], bc[:])
                return cos_sc, sin_sc, istd_bc

            # ---- stage B: QKV projections + rope / V-natural; the gram
            # matmuls for chunk c+1 are emitted between the q and k groups
            # so their DVE diag-extract has a whole projection group of
            # slack before the attention sp tiles need the PSUM slot ----
            def emit_b(c, ht, stats, ht_next):
                cos_sc, sin_sc, istd_bc = stats
                qt_c = []
                istd4 = None
                for kind, w_sb in (("q", wq_sb), ("k", wk_sb), ("v", wv_sb)):
                    pp = big.tile([128, 1024], FP32, name="pp", tag="big")
                    for h in range(HPC):
                        for dt in range(DT):
                            nc.tensor.matmul(
                                pp[:, h * 512:(h + 1) * 512],
                                w_sb[:, dt * M + h * HD:dt * M + h * HD + 128],
                                ht[:, dt * CHUNK:(dt + 1) * CHUNK],
                                start=(dt == 0), stop=(dt == DT - 1))
                    if kind == "v":
                        vstage = vstp.tile([128, 1024], BF16, name="vstage")
                        for h in range(HPC):
                            nc.vector.tensor_tensor(
                                out=vstage[:, h * 512:(h + 1) * 512],
                                in0=pp[:, h * 512:(h + 1) * 512],
                                in1=istd_bc[:], op=ALU.mult)
                        vt = big.tile([128, 1024], FP32, name="vt", tag="big")
                        for h in range(HPC):
                            for b in range(4):
                                sl0 = h * 512 + b * 128
                                nc.tensor.matmul(
                                    vt[:, sl0:sl0 + 128],
                                    vstage[:, sl0:sl0 + 128], ident_sb[:],
                                    start=True, stop=True)
                        for h in range(HPC):
                            nc.vector.tensor_copy(
                                vn[h][:, c * CHUNK:(c + 1) * CHUNK],
                                vt[:, h * 512:(h + 1) * 512])
                    else:
                        for h in range(HPC):
                            hsl = slice(h * 512, (h + 1) * 512)
                            if kind == "q":
                                dst_t = qtcp.tile([128, CHUNK], BF16,
                                                  name=f"qt{h}")
                                qt_c.append(dst_t)
                                dst = dst_t[:]
                            else:
                                dst = kt[h][:, c * CHUNK:(c + 1) * CHUNK]
                            pc = rpp.tile([128, CHUNK], FP32, name="pc")
                            nc.vector.tensor_tensor(
                                out=pc[:], in0=pp[:, hsl],
                                in1=cos_sc[:], op=ALU.mult)
                            psw = rpp.tile([128, CHUNK], FP32, name="psw")
                            nc.vector.tensor_tensor(
                                out=psw[0:64, :], in0=pp[64:128, hsl],
                                in1=sin_sc[0:64, :], op=ALU.mult)
                            nc.vector.tensor_tensor(
                                out=psw[64:128, :], in0=pp[0:64, hsl],
                                in1=sin_sc[64:128, :], op=ALU.mult)
                            nc.vector.tensor_tensor(
                                out=dst, in0=pc[:], in1=psw[:], op=ALU.add)
                return qt_c, istd4

            # ---- stage C: attention core for one head (returns oz) ----
            def emit_c_core(c, h, qt_c):
                npair = 2 * c + 2
                jmax = 4 * c + 3
                LAGP = 3
                oz = ozp.tile([128, 1024], FP32, name="oz", tag="oz")
                pend = {}
                for p in range(npair + LAGP):
                    if p < npair:
                        sp = big.tile([128, 1024], FP32, name="sp", tag="big")
                        for i in range(2):
                            j = 2 * p + i
                            off = max(j - 4 * c, 0) * 128
                            nc.tensor.matmul(
                                sp[:, i * 512 + off:(i + 1) * 512],
                                kt[h][:, j * 128:(j + 1) * 128],
                                qt_c[h][:, off:], start=True, stop=True)
                        pt = ptp.tile([128, 1024], BF16, name="pt")
                        nc.scalar.activation(pt[:], sp[:], AF.Exp,
                                             scale=float(SM_SCALE))
                        for i in range(2):
                            r = 2 * p + i - 4 * c
                            if r >= 0:
                                off = i * 512 + r * 128
                                nc.vector.tensor_tensor(
                                    out=pt[:, off:off + 128],
                                    in0=pt[:, off:off + 128],
                                    in1=tri_sb[:], op=ALU.mult)
                        pend[p] = pt
                    if p >= LAGP:
                        pt = pend.pop(p - LAGP)
                        for i in range(2):
                            j = 2 * (p - LAGP) + i
                            off = max(j - 4 * c, 0) * 128
                            nc.tensor.matmul(
                                oz[:, 512 + off:1024], ones_sb[:],
                                pt[:, i * 512 + off:(i + 1) * 512],
                                start=(j == 0), stop=(j == jmax),
                                skip_group_check=True)
                            nc.tensor.matmul(
                                oz[:, off:512],
                                vn[h][:, j * 128:(j + 1) * 128],
                                pt[:, i * 512 + off:(i + 1) * 512],
                                start=(j == 0), stop=(j == jmax),
                                skip_group_check=True)
                return oz

            # ---- softmax normalize, 128-col blocks so stage D can start
            # on block 0 without waiting for a full-width reciprocal ----
            def emit_c_norm(h, oz):
                at = attp.tile([128, CHUNK], BF16, name=f"at{h}")
                rz = rzp.tile([128, CHUNK], FP32, name=f"rz{h}")
                for b in range(4):
                    bsl = slice(b * 128, (b + 1) * 128)
                    nc.vector.reciprocal(rz[:, bsl], oz[:, 512 + b * 128:
                                                        512 + (b + 1) * 128])
                    nc.vector.tensor_tensor(out=at[:, bsl],
                                            in0=oz[:, b * 128:(b + 1) * 128],
                                            in1=rz[:, bsl], op=ALU.mult)
                return at

            # ---- stage D: output projection for chunk c ----
            def emit_d(c, ats):
                # op tiles live in the oz pool (idle during D) so the big
                # pool is free for the next chunk's QKV to start while the
                # D drains are still in flight. Out DMAs alternate queues.
                for st4 in range(4):
                    st = 4 * c + st4
                    ost = ostp.tile([128, D], FP32, name="ost")
                    for dq in range(2):
                        op = ozp.tile([128, 1024], FP32, name="op", tag="oz")
                        for hh in range(2):
                            for k2 in range(2):
                                dc = dq * 2 + k2
                                dsl = slice(dc * 512, (dc + 1) * 512)
                                nc.tensor.matmul(
                                    op[:, k2 * 512:(k2 + 1) * 512],
                                    ats[hh][:, st4 * 128:(st4 + 1) * 128],
                                    wot[hh][:, dsl], start=(hh == 0),
                                    stop=(hh == 1))
                        if dq == 0:
                            nc.scalar.activation(
                                ost[:, dq * 1024:(dq + 1) * 1024], op[:],
                                AF.Copy)
                        else:
                            nc.vector.tensor_copy(
                                ost[:, dq * 1024:(dq + 1) * 1024], op[:])
                    eng = nc.sync if st4 % 2 == 0 else nc.scalar
                    eng.dma_start(out=out[st * 128:(st + 1) * 128, :],
                                  in_=ost[:])

            def emit_keepwarm(n):
                for _ in range(n):
                    wm = big.tile([128, 512], FP32, name="warm", tag="big")
                    nc.tensor.matmul(wm[:], junk[:, 0:128], junk[:],
                                     start=True, stop=True)

            # ---- wot build (wo^T per head) ----
            def emit_wot():
                for h in range(HPC):
                    for q in range(2):
                        wp = big.tile([128, 1024], FP32, name="wp", tag="big")
                        for dl in range(8):
                            dt = q * 8 + dl
                            nc.tensor.matmul(
                                wp[:, dl * 128:(dl + 1) * 128],
                                wo_sb[:, dt * M + h * HD:dt * M + h * HD + 128],
                                ident_sb[:], start=True, stop=True)
                        nc.vector.tensor_copy(
                            wot[h][:, q * 1024:(q + 1) * 1024], wp[:])

            # ---- main fused loop ----
            # stats PE ops (itT/bcast) for chunk c+1 are emitted between the
            # two attention heads of chunk c so their DVE inputs are long
            # resolved and the PE never stalls on them.
            istd4 = emit_stats_gram(0, ht_cur)
            stats = emit_stats_bcast(0, istd4)
            pend_d = None
            for c in range(NCHUNK):
                ht_next = emit_ht_dma(c + 1) if c + 1 < NCHUNK else None
                qt_c, istd4 = emit_b(c, ht_cur, stats, None)
                if pend_d is not None:
                    emit_d(*pend_d)
                if ht_next is not None:
                    istd4 = emit_stats_gram(c + 1, ht_next)
                oz0 = emit_c_core(c, 0, qt_c)
                if ht_next is not None:
                    stats = emit_stats_bcast(c + 1, istd4)
                if c == 0:
                    emit_wot()
                at0 = emit_c_norm(0, oz0)
                oz1 = emit_c_core(c, 1, qt_c)
                at1 = emit_c_norm(1, oz1)
                pend_d = (c, [at0, at1])
                if c < 3:
                    emit_keepwarm(10 - 3 * c)
                ht_cur = ht_next
            emit_d(*pend_d)

    nc.finalize()
    return nc


def _host_prep(xs, norm_w, wq, wk, wv, wo):
    """Fold norm_w into qkv weights, permute rope dims, build tables."""
    bf16 = ml_dtypes.bfloat16
    nw = norm_w.astype(np.float32)[:, None, None]
    perm = np.concatenate([np.arange(0, HD, 2), np.arange(1, HD, 2)])
    wq_p = (wq * nw)[:, :, perm]
    wk_p = (wk * nw)[:, :, perm]
    wv_n = wv * nw

    inv_freq = 1.0 / (ROPE_BASE ** (np.arange(0, HD, 2, dtype=np.float32) / HD))
    pos = np.arange(SEQ, dtype=np.float32)
    ang = pos[:, None] * inv_freq[None, :]          # [S, 64]
    cos_t = np.cos(ang).T.astype(np.float32)        # [64, S]
    sin_t = np.sin(ang).T.astype(np.float32)
    cosd = np.concatenate([cos_t, cos_t], 0)        # [128, S]
    # [-sin; sin]: dst = pp*cos_sc + psw, psw[0:64] = pp[64:]*(-sin*istd),
    # psw[64:] = pp[0:64]*(sin*istd)
    sind = np.concatenate([-sin_t, sin_t], 0)

    tri = np.triu(np.ones((128, 128), dtype=np.float32))  # t <= s valid
    onesm = np.ones((128, 128), dtype=np.float32)
    identm = np.eye(128, dtype=np.float32)
    sel = np.kron(np.eye(4, dtype=np.float32), np.ones((1, 128), np.float32))

    common = {
        "xsT": np.ascontiguousarray(xs.astype(np.float32).T.astype(bf16)),
        "cosd": np.ascontiguousarray(cosd.astype(bf16)),
        "sind": np.ascontiguousarray(sind.astype(bf16)),
        "tri": np.ascontiguousarray(tri.astype(bf16)),
        "ones": onesm.astype(bf16),
        "ident": identm.astype(bf16),
        "identr": identm,
        "sel4": np.ascontiguousarray(sel),
    }
    in_maps = []
    for core in range(NCORES):
        h0 = core * HPC
        sl = slice(h0, h0 + HPC)
        in_maps.append({
            **common,
            "wq": np.ascontiguousarray(
                wq_p[:, sl, :].reshape(D, M).astype(bf16)),
            "wk": np.ascontiguousarray(
                wk_p[:, sl, :].reshape(D, M).astype(bf16)),
            "wv": np.ascontiguousarray(
                wv_n[:, sl, :].reshape(D, M).astype(bf16)),
            "wo": np.ascontiguousarray(
                wo[:, sl, :].reshape(D, M).astype(bf16)),
        })
    return in_maps


def kernel(xs, norm_w, wq, wk, wv, wo):
    trace = bool(int(os.environ.get("KERNEL_TRACE", "0")))
    if trace:
        _inject_ntff_hook()
    from concourse.bass_utils import run_bass_kernel_spmd

    nc = _build_nc()
    in_maps = _host_prep(np.asarray(xs), np.asarray(norm_w), np.asarray(wq),
                         np.asarray(wk), np.asarray(wv), np.asarray(wo))
    res = run_bass_kernel_spmd(nc, in_maps, core_ids=list(range(NCORES)),
                               trace=trace)
    if trace and res.exec_time_ns is not None:
        print(f"HW exec time: {res.exec_time_ns} ns")
    acc = np.zeros((SEQ, D), dtype=np.float64)
    for r in res.results:
        acc += r["out"].astype(np.float64)
    return acc.astype(np.float32)


if __name__ == "__main__":
    rng = np.random.default_rng(0)
    scale = 1.0 / np.sqrt(D)
    inputs = {
        "xs": rng.standard_normal((SEQ, D), dtype=np.float32),
        "norm_w": np.ones((D,), np.float32),
        "wq": rng.standard_normal((D, NH, HD), dtype=np.float32) * scale,
        "wk": rng.standard_normal((D, NH, HD), dtype=np.float32) * scale,
        "wv": rng.standard_normal((D, NH, HD), dtype=np.float32) * scale,
        "wo": rng.standard_normal((D, NH, HD), dtype=np.float32) * scale,
    }
    out = kernel(**inputs)
    print(out.shape, out.dtype, float(np.abs(out).max()))


# revision 28
# speedup vs baseline: 1.0572x; 1.0572x over previous
"""Trainium2 Bass kernel: RMSNorm + RoPE + causal attention + output projection.

Tensor-parallel over heads: 16 heads / 8 cores = 2 heads per core.
Each core computes a full [S, D] partial output; the all-reduce is done
host-side in the gather.

v3 design (all-bf16 matmuls, host-transposed activations, fused single pass):
  - xs is transposed on the host: x^T [D, S] bf16 is DMA'd straight into
    SBUF per chunk — no PE transposes of activations, no PSUM->SBUF copies
    for h^T at all.
  - RMSNorm: sum-of-squares via a PE gram accumulation on the x^T tiles
    (diagonal extracted with one fused DVE op per s-tile), and the istd
    scale folded into per-chunk scaled rope tables (Q,K) and an istd
    multiply on V^T. h^T itself stays unscaled.
  - istd broadcast [s] -> [128, s-chunk] via a tiny fp32r transpose plus
    4 selector matmuls (proven in probe).
  - All PE matmuls bf16 (FWL fast weight loads), fp32 PSUM accumulation.
  - Attention per head with paired key-tiles: exp batched [128,1024] on
    the scalar engine, Z via ones-matmul, PV accumulation, LAG-2 software
    pipeline; causal handled by full-width diagonal scores + triangular
    mask + column-offset slicing of Z/PV.
  - Output projection inlined per chunk, drains split scalar/DVE.
  - PE warmup matmuls on junk data cover the initial DMA window.
"""
import os
import sys
import types

import numpy as np
import ml_dtypes

SEQ, D, NH, HD = 4096, 2048, 16, 128
NCORES = 8
HPC = NH // NCORES          # heads per core
M = HPC * HD                # per-core fused head dim (256)
EPS = 1e-6
ROPE_BASE = 10000.0
SM_SCALE = 1.0 / np.sqrt(HD)
CHUNK = 512                 # q-chunk
NCHUNK = SEQ // CHUNK       # 8
DT = D // 128               # 16 d-tiles
NWARM = 44                  # PE warmup matmuls during initial DMA window


def _inject_ntff_hook():
    """Register the axon NTFF profiling hook (missing antenv.axon_hooks)."""
    if "antenv.axon_hooks" in sys.modules:
        return
    try:
        import antenv
        from trn_agent_boot.trn_boot import _ntff_profile_via_ctypes
    except ImportError:
        return
    holder = [None]
    mod = types.ModuleType("antenv.axon_hooks")
    mod.set_axon_ntff_profile_hook = lambda h: holder.__setitem__(0, h)
    mod.get_axon_ntff_profile_hook = lambda: holder[0]
    sys.modules["antenv.axon_hooks"] = mod
    antenv.axon_hooks = mod
    try:
        mod.set_axon_ntff_profile_hook(
            _ntff_profile_via_ctypes("/opt/axon/libaxon_pjrt.so"))
    except Exception:
        pass


def _build_nc():
    import concourse.bass as bass  # noqa: F401
    import concourse.mybir as mybir
    import concourse.tile as tile
    from concourse import bacc

    FP32 = mybir.dt.float32
    FP32R = mybir.dt.float32r
    BF16 = mybir.dt.bfloat16
    U32 = mybir.dt.uint32
    AF = mybir.ActivationFunctionType
    ALU = mybir.AluOpType

    nc = bacc.Bacc(None, target_bir_lowering=False)

    xsT = nc.declare_dram_parameter("xsT", [D, SEQ], BF16, isOutput=False)
    wq = nc.declare_dram_parameter("wq", [D, M], BF16, isOutput=False)
    wk = nc.declare_dram_parameter("wk", [D, M], BF16, isOutput=False)
    wv = nc.declare_dram_parameter("wv", [D, M], BF16, isOutput=False)
    wo = nc.declare_dram_parameter("wo", [D, M], BF16, isOutput=False)
    cosd = nc.declare_dram_parameter("cosd", [128, SEQ], BF16, isOutput=False)
    sind = nc.declare_dram_parameter("sind", [128, SEQ], BF16, isOutput=False)
    tri = nc.declare_dram_parameter("tri", [128, 128], BF16, isOutput=False)
    ones = nc.declare_dram_parameter("ones", [128, 128], BF16, isOutput=False)
    ident = nc.declare_dram_parameter("ident", [128, 128], BF16, isOutput=False)
    identr = nc.declare_dram_parameter("identr", [128, 128], FP32R,
                                       isOutput=False)
    sel4 = nc.declare_dram_parameter("sel4", [4, 512], FP32R, isOutput=False)
    out = nc.declare_dram_parameter("out", [SEQ, D], FP32, isOutput=True)

    with tile.TileContext(nc) as tc:
        with tc.tile_pool(name="consts", bufs=1) as consts, \
             tc.tile_pool(name="state", bufs=1) as state, \
             tc.tile_pool(name="ht", bufs=2) as htp, \
             tc.tile_pool(name="nrm", bufs=2) as nrm, \
             tc.tile_pool(name="sct", bufs=2) as sct, \
             tc.tile_pool(name="qtc", bufs=2) as qtcp, \
             tc.tile_pool(name="vst", bufs=2) as vstp, \
             tc.tile_pool(name="rp", bufs=2) as rpp, \
             tc.tile_pool(name="pt", bufs=5) as ptp, \
             tc.tile_pool(name="rz", bufs=2) as rzp, \
             tc.tile_pool(name="att", bufs=2) as attp, \
             tc.tile_pool(name="ost", bufs=2) as ostp, \
             tc.tile_pool(name="big", bufs=2, space="PSUM") as big, \
             tc.tile_pool(name="oz", bufs=2, space="PSUM") as ozp:

            # ---- warmup: keep PE busy during the initial DMA window ----
            junk = consts.tile([128, 512], BF16)
            nc.vector.memset(junk[:], 0.125)
            for _ in range(NWARM):
                wm = big.tile([128, 512], FP32, name="warm", tag="big")
                nc.tensor.matmul(wm[:], junk[:, 0:128], junk[:],
                                 start=True, stop=True)

            # ---- setup DMAs (consts on the vector queue; activations and
            # weights on sync/scalar queues so they don't serialize) ----
            ident_sb = consts.tile([128, 128], BF16)
            nc.scalar.dma_start(out=ident_sb[:], in_=ident[:])
            idr_sb = consts.tile([128, 128], FP32R)
            nc.scalar.dma_start(out=idr_sb[:], in_=identr[:])
            sel_sb = consts.tile([4, 512], FP32R)
            nc.scalar.dma_start(out=sel_sb[:], in_=sel4[:])
            magic_sb = consts.tile([128, 4], U32)
            nc.vector.memset(magic_sb[:], 0x5F3759DF)

            # persistent per-head state
            kt = [state.tile([128, SEQ], BF16, name=f"kt{h}") for h in range(HPC)]
            vn = [state.tile([128, SEQ], BF16, name=f"vn{h}") for h in range(HPC)]
            wot = [state.tile([128, D], BF16, name=f"wot{h}") for h in range(HPC)]

            def emit_ht_dma(c):
                ht = htp.tile([128, DT * CHUNK], BF16, name="ht")
                nc.sync.dma_start(
                    out=ht[:].rearrange("p (t s) -> p t s", t=DT),
                    in_=xsT[:, c * CHUNK:(c + 1) * CHUNK].rearrange(
                        "(t p) s -> p t s", p=128))
                return ht

            # chunk 0 activations first, then weights/tables
            ht_cur = emit_ht_dma(0)

            wq_sb = consts.tile([128, DT * M], BF16)
            nc.sync.dma_start(out=wq_sb[:].rearrange("p (t m) -> p t m", t=DT),
                              in_=wq[:].rearrange("(t p) m -> p t m", p=128))
            wk_sb = consts.tile([128, DT * M], BF16)
            nc.sync.dma_start(out=wk_sb[:].rearrange("p (t m) -> p t m", t=DT),
                              in_=wk[:].rearrange("(t p) m -> p t m", p=128))
            wv_sb = consts.tile([128, DT * M], BF16)
            nc.sync.dma_start(out=wv_sb[:].rearrange("p (t m) -> p t m", t=DT),
                              in_=wv[:].rearrange("(t p) m -> p t m", p=128))
            cos_sb = consts.tile([128, SEQ], BF16)
            nc.scalar.dma_start(out=cos_sb[:], in_=cosd[:])
            sin_sb = consts.tile([128, SEQ], BF16)
            nc.scalar.dma_start(out=sin_sb[:], in_=sind[:])
            tri_sb = consts.tile([128, 128], BF16)
            nc.scalar.dma_start(out=tri_sb[:], in_=tri[:])
            ones_sb = consts.tile([128, 128], BF16)
            nc.scalar.dma_start(out=ones_sb[:], in_=ones[:])
            wo_sb = consts.tile([128, DT * M], BF16)
            nc.scalar.dma_start(out=wo_sb[:].rearrange("p (t m) -> p t m", t=DT),
                                in_=wo[:].rearrange("(t p) m -> p t m", p=128))

            # ---- norm stats, DVE part: gram diag -> istd4 (bit-hack rsqrt,
            # no scalar-engine op => no activation-table thrash) ----
            def emit_stats_gram(c, ht):
                gram = big.tile([128, 512], FP32, name="gram", tag="big")
                for st in range(4):
                    for dt in range(DT):
                        blk = ht[:, dt * CHUNK + st * 128:
                                 dt * CHUNK + (st + 1) * 128]
                        nc.tensor.matmul(gram[:, st * 128:(st + 1) * 128],
                                         blk, blk,
                                         start=(dt == 0), stop=(dt == DT - 1))
                scr = nrm.tile([128, 128], BF16, name="scr")
                ssq4 = nrm.tile([128, 4], FP32, name="ssq4")
                for st in range(4):
                    nc.vector.scalar_tensor_tensor(
                        out=scr[:], in0=gram[:, st * 128:(st + 1) * 128],
                        scalar=1.0, in1=ident_sb[:],
                        op0=ALU.mult, op1=ALU.mult,
                        accum_out=ssq4[:, st:st + 1])
                # istd = rsqrt(ssq/D + eps): bit-hack + 2 Newton iterations
                ms = nrm.tile([128, 4], FP32, name="ms")
                nc.vector.tensor_scalar(out=ms[:], in0=ssq4[:],
                                        scalar1=1.0 / D, scalar2=EPS,
                                        op0=ALU.mult, op1=ALU.add)
                ih = nrm.tile([128, 4], U32, name="ih")
                nc.vector.tensor_scalar(out=ih[:], in0=ms[:].bitcast(U32),
                                        scalar1=1, scalar2=None,
                                        op0=ALU.logical_shift_right)
                y = nrm.tile([128, 4], FP32, name="y")
                nc.vector.scalar_tensor_tensor(
                    out=y[:].bitcast(U32), in0=magic_sb[:], scalar=0,
                    in1=ih[:], op0=ALU.bypass, op1=ALU.subtract)
                t = nrm.tile([128, 4], FP32, name="t")
                istd4 = nrm.tile([128, 4], FP32R, name="istd4")
                for it in range(2):
                    nc.vector.tensor_tensor(out=t[:], in0=y[:], in1=y[:],
                                            op=ALU.mult)
                    nc.vector.tensor_tensor(out=t[:], in0=t[:], in1=ms[:],
                                            op=ALU.mult)
                    nc.vector.tensor_scalar(out=t[:], in0=t[:], scalar1=-0.5,
                                            scalar2=1.5, op0=ALU.mult,
                                            op1=ALU.add)
                    dst = y[:] if it == 0 else istd4[:]
                    with nc.allow_low_precision(reason="istd bcast chain"):
                        nc.vector.tensor_tensor(out=dst, in0=y[:], in1=t[:],
                                                op=ALU.mult)
                return istd4

            # ---- norm stats, PE part: broadcast istd + scaled tables ----
            def emit_stats_bcast(c, istd4):
                csl = slice(c * CHUNK, (c + 1) * CHUNK)
                itT = big.tile([4, 128], FP32R, name="itT", tag="big")
                nc.tensor.transpose(itT[:], istd4[:], idr_sb[:])
                itT_sb = nrm.tile([4, 128], FP32R, name="itT_sb")
                nc.vector.tensor_copy(itT_sb[:], itT[:])
                bc = big.tile([128, 512], FP32, name="bc", tag="big")
                for st in range(4):
                    nc.tensor.matmul(bc[:, st * 128:(st + 1) * 128],
                                     sel_sb[:, st * 128:(st + 1) * 128],
                                     itT_sb[:], start=True, stop=True)
                cos_sc = sct.tile([128, CHUNK], BF16, name="cos_sc")
                nc.vector.tensor_tensor(out=cos_sc[:], in0=bc[:],
                                        in1=cos_sb[:, csl], op=ALU.mult)
                sin_sc = sct.tile([128, CHUNK], BF16, name="sin_sc")
                nc.vector.tensor_tensor(out=sin_sc[:], in0=bc[:],
                                        in1=sin_sb[:, csl], op=ALU.mult)
                return cos_sc, sin_sc, istd4

            # ---- stage B: QKV projections + rope / V-natural; the gram
            # matmuls for chunk c+1 are emitted between the q and k groups
            # so their DVE diag-extract has a whole projection group of
            # slack before the attention sp tiles need the PSUM slot ----
            def emit_b(c, ht, stats, ht_next):
                cos_sc, sin_sc, istd4c = stats
                qt_c = []
                istd4 = None
                for kind, w_sb in (("q", wq_sb), ("k", wk_sb), ("v", wv_sb)):
                    pp = big.tile([128, 1024], FP32, name="pp", tag="big")
                    for h in range(HPC):
                        for dt in range(DT):
                            nc.tensor.matmul(
                                pp[:, h * 512:(h + 1) * 512],
                                w_sb[:, dt * M + h * HD:dt * M + h * HD + 128],
                                ht[:, dt * CHUNK:(dt + 1) * CHUNK],
                                start=(dt == 0), stop=(dt == DT - 1))
                    if kind == "v":
                        vstage = vstp.tile([128, 1024], BF16, name="vstage")
                        nc.scalar.activation(vstage[:], pp[:], AF.Copy)
                        vt = big.tile([128, 1024], FP32, name="vt", tag="big")
                        for h in range(HPC):
                            for b in range(4):
                                sl0 = h * 512 + b * 128
                                nc.tensor.matmul(
                                    vt[:, sl0:sl0 + 128],
                                    vstage[:, sl0:sl0 + 128], ident_sb[:],
                                    start=True, stop=True)
                        # istd_t applied per-partition during the PSUM drain
                        for h in range(HPC):
                            for b in range(4):
                                nc.vector.tensor_scalar_mul(
                                    vn[h][:, (4 * c + b) * 128:
                                          (4 * c + b + 1) * 128],
                                    vt[:, h * 512 + b * 128:
                                       h * 512 + (b + 1) * 128],
                                    istd4c[:, b:b + 1].bitcast(
                                        mybir.dt.float32))
                    else:
                        for h in range(HPC):
                            hsl = slice(h * 512, (h + 1) * 512)
                            if kind == "q":
                                dst_t = qtcp.tile([128, CHUNK], BF16,
                                                  name=f"qt{h}")
                                qt_c.append(dst_t)
                                dst = dst_t[:]
                            else:
                                dst = kt[h][:, c * CHUNK:(c + 1) * CHUNK]
                            pc = rpp.tile([128, CHUNK], FP32, name="pc")
                            nc.vector.tensor_tensor(
                                out=pc[:], in0=pp[:, hsl],
                                in1=cos_sc[:], op=ALU.mult)
                            psw = rpp.tile([128, CHUNK], FP32, name="psw")
                            nc.vector.tensor_tensor(
                                out=psw[0:64, :], in0=pp[64:128, hsl],
                                in1=sin_sc[0:64, :], op=ALU.mult)
                            nc.vector.tensor_tensor(
                                out=psw[64:128, :], in0=pp[0:64, hsl],
                                in1=sin_sc[64:128, :], op=ALU.mult)
                            nc.vector.tensor_tensor(
                                out=dst, in0=pc[:], in1=psw[:], op=ALU.add)
                return qt_c, istd4

            # ---- stage C: attention core for one head (returns oz) ----
            def emit_c_core(c, h, qt_c):
                npair = 2 * c + 2
                jmax = 4 * c + 3
                LAGP = 3
                oz = ozp.tile([128, 1024], FP32, name="oz", tag="oz")
                pend = {}
                for p in range(npair + LAGP):
                    if p < npair:
                        sp = big.tile([128, 1024], FP32, name="sp", tag="big")
                        for i in range(2):
                            j = 2 * p + i
                            off = max(j - 4 * c, 0) * 128
                            nc.tensor.matmul(
                                sp[:, i * 512 + off:(i + 1) * 512],
                                kt[h][:, j * 128:(j + 1) * 128],
                                qt_c[h][:, off:], start=True, stop=True)
                        pt = ptp.tile([128, 1024], BF16, name="pt")
                        nc.scalar.activation(pt[:], sp[:], AF.Exp,
                                             scale=float(SM_SCALE))
                        for i in range(2):
                            r = 2 * p + i - 4 * c
                            if r >= 0:
                                off = i * 512 + r * 128
                                nc.vector.tensor_tensor(
                                    out=pt[:, off:off + 128],
                                    in0=pt[:, off:off + 128],
                                    in1=tri_sb[:], op=ALU.mult)
                        pend[p] = pt
                    if p >= LAGP:
                        pt = pend.pop(p - LAGP)
                        for i in range(2):
                            j = 2 * (p - LAGP) + i
                            off = max(j - 4 * c, 0) * 128
                            nc.tensor.matmul(
                                oz[:, 512 + off:1024], ones_sb[:],
                                pt[:, i * 512 + off:(i + 1) * 512],
                                start=(j == 0), stop=(j == jmax),
                                skip_group_check=True)
                            nc.tensor.matmul(
                                oz[:, off:512],
                                vn[h][:, j * 128:(j + 1) * 128],
                                pt[:, i * 512 + off:(i + 1) * 512],
                                start=(j == 0), stop=(j == jmax),
                                skip_group_check=True)
                return oz

            # ---- softmax normalize, 128-col blocks so stage D can start
            # on block 0 without waiting for a full-width reciprocal ----
            def emit_c_norm(h, oz):
                at = attp.tile([128, CHUNK], BF16, name=f"at{h}")
                rz = rzp.tile([128, CHUNK], FP32, name=f"rz{h}")
                for b in range(4):
                    bsl = slice(b * 128, (b + 1) * 128)
                    nc.vector.reciprocal(rz[:, bsl], oz[:, 512 + b * 128:
                                                        512 + (b + 1) * 128])
                    nc.vector.tensor_tensor(out=at[:, bsl],
                                            in0=oz[:, b * 128:(b + 1) * 128],
                                            in1=rz[:, bsl], op=ALU.mult)
                return at

            # ---- stage D: output projection for chunk c ----
            def emit_d(c, ats, split_dma=False):
                # op tiles live in the oz pool (idle during D) so the big
                # pool is free for the next chunk's QKV to start while the
                # D drains are still in flight. Out DMAs alternate queues.
                for st4 in range(4):
                    st = 4 * c + st4
                    ost = ostp.tile([128, D], FP32, name="ost")
                    for dq in range(2):
                        op = ozp.tile([128, 1024], FP32, name="op", tag="oz")
                        for hh in range(2):
                            for k2 in range(2):
                                dc = dq * 2 + k2
                                dsl = slice(dc * 512, (dc + 1) * 512)
                                nc.tensor.matmul(
                                    op[:, k2 * 512:(k2 + 1) * 512],
                                    ats[hh][:, st4 * 128:(st4 + 1) * 128],
                                    wot[hh][:, dsl], start=(hh == 0),
                                    stop=(hh == 1))
                        if dq == 0:
                            nc.scalar.activation(
                                ost[:, dq * 1024:(dq + 1) * 1024], op[:],
                                AF.Copy)
                        else:
                            nc.vector.tensor_copy(
                                ost[:, dq * 1024:(dq + 1) * 1024], op[:])
                        if split_dma:
                            eng = nc.sync if (st4 + dq) % 2 == 0 else nc.scalar
                            eng.dma_start(
                                out=out[st * 128:(st + 1) * 128,
                                        dq * 1024:(dq + 1) * 1024],
                                in_=ost[:, dq * 1024:(dq + 1) * 1024])
                    if not split_dma:
                        eng = nc.sync if st4 % 2 == 0 else nc.scalar
                        eng.dma_start(out=out[st * 128:(st + 1) * 128, :],
                                      in_=ost[:])

            def emit_keepwarm(n):
                wm = big.tile([128, 512], FP32, name="warm", tag="big")
                for _ in range(n):
                    nc.tensor.matmul(wm[:], junk[:, 0:128], junk[:],
                                     start=True, stop=True)

            # ---- wot build (wo^T per head) ----
            def emit_wot():
                for h in range(HPC):
                    for q in range(2):
                        wp = big.tile([128, 1024], FP32, name="wp", tag="big")
                        for dl in range(8):
                            dt = q * 8 + dl
                            nc.tensor.matmul(
                                wp[:, dl * 128:(dl + 1) * 128],
                                wo_sb[:, dt * M + h * HD:dt * M + h * HD + 128],
                                ident_sb[:], start=True, stop=True)
                        nc.vector.tensor_copy(
                            wot[h][:, q * 1024:(q + 1) * 1024], wp[:])

            # ---- main fused loop ----
            # stats PE ops (itT/bcast) for chunk c+1 are emitted between the
            # two attention heads of chunk c so their DVE inputs are long
            # resolved and the PE never stalls on them.
            istd4 = emit_stats_gram(0, ht_cur)
            emit_keepwarm(14)
            stats = emit_stats_bcast(0, istd4)
            for c in range(NCHUNK):
                ht_next = emit_ht_dma(c + 1) if c + 1 < NCHUNK else None
                qt_c, istd4 = emit_b(c, ht_cur, stats, None)
                if ht_next is not None:
                    istd4 = emit_stats_gram(c + 1, ht_next)
                oz0 = emit_c_core(c, 0, qt_c)
                if ht_next is not None:
                    stats = emit_stats_bcast(c + 1, istd4)
                if c == 0:
                    emit_wot()
                at0 = emit_c_norm(0, oz0)
                oz1 = emit_c_core(c, 1, qt_c)
                at1 = emit_c_norm(1, oz1)
                emit_d(c, [at0, at1], split_dma=(c == NCHUNK - 1))
                if c < 3:
                    emit_keepwarm(10 - 3 * c)
                ht_cur = ht_next

    nc.finalize()
    return nc


def _host_prep(xs, norm_w, wq, wk, wv, wo):
    """Fold norm_w into qkv weights, permute rope dims, build tables."""
    bf16 = ml_dtypes.bfloat16
    nw = norm_w.astype(np.float32)[:, None, None]
    perm = np.concatenate([np.arange(0, HD, 2), np.arange(1, HD, 2)])
    wq_p = (wq * nw)[:, :, perm]
    wk_p = (wk * nw)[:, :, perm]
    wv_n = wv * nw

    inv_freq = 1.0 / (ROPE_BASE ** (np.arange(0, HD, 2, dtype=np.float32) / HD))
    pos = np.arange(SEQ, dtype=np.float32)
    ang = pos[:, None] * inv_freq[None, :]          # [S, 64]
    cos_t = np.cos(ang).T.astype(np.float32)        # [64, S]
    sin_t = np.sin(ang).T.astype(np.float32)
    cosd = np.concatenate([cos_t, cos_t], 0)        # [128, S]
    # [-sin; sin]: dst = pp*cos_sc + psw, psw[0:64] = pp[64:]*(-sin*istd),
    # psw[64:] = pp[0:64]*(sin*istd)
    sind = np.concatenate([-sin_t, sin_t], 0)

    tri = np.triu(np.ones((128, 128), dtype=np.float32))  # t <= s valid
    onesm = np.ones((128, 128), dtype=np.float32)
    identm = np.eye(128, dtype=np.float32)
    sel = np.kron(np.eye(4, dtype=np.float32), np.ones((1, 128), np.float32))

    common = {
        "xsT": np.ascontiguousarray(xs.astype(np.float32).T.astype(bf16)),
        "cosd": np.ascontiguousarray(cosd.astype(bf16)),
        "sind": np.ascontiguousarray(sind.astype(bf16)),
        "tri": np.ascontiguousarray(tri.astype(bf16)),
        "ones": onesm.astype(bf16),
        "ident": identm.astype(bf16),
        "identr": identm,
        "sel4": np.ascontiguousarray(sel),
    }
    in_maps = []
    for core in range(NCORES):
        h0 = core * HPC
        sl = slice(h0, h0 + HPC)
        in_maps.append({
            **common,
            "wq": np.ascontiguousarray(
                wq_p[:, sl, :].reshape(D, M).astype(bf16)),
            "wk": np.ascontiguousarray(
                wk_p[:, sl, :].reshape(D, M).astype(bf16)),
            "wv": np.ascontiguousarray(
                wv_n[:, sl, :].reshape(D, M).astype(bf16)),
            "wo": np.ascontiguousarray(
                wo[:, sl, :].reshape(D, M).astype(bf16)),
        })
    return in_maps


def kernel(xs, norm_w, wq, wk, wv, wo):
    trace = bool(int(os.environ.get("KERNEL_TRACE", "0")))
    if trace:
        _inject_ntff_hook()
    from concourse.bass_utils import run_bass_kernel_spmd

    nc = _build_nc()
    in_maps = _host_prep(np.asarray(xs), np.asarray(norm_w), np.asarray(wq),
                         np.asarray(wk), np.asarray(wv), np.asarray(wo))
    res = run_bass_kernel_spmd(nc, in_maps, core_ids=list(range(NCORES)),
                               trace=trace)
    if trace and res.exec_time_ns is not None:
        print(f"HW exec time: {res.exec_time_ns} ns")
    acc = np.zeros((SEQ, D), dtype=np.float64)
    for r in res.results:
        acc += r["out"].astype(np.float64)
    return acc.astype(np.float32)


if __name__ == "__main__":
    rng = np.random.default_rng(0)
    scale = 1.0 / np.sqrt(D)
    inputs = {
        "xs": rng.standard_normal((SEQ, D), dtype=np.float32),
        "norm_w": np.ones((D,), np.float32),
        "wq": rng.standard_normal((D, NH, HD), dtype=np.float32) * scale,
        "wk": rng.standard_normal((D, NH, HD), dtype=np.float32) * scale,
        "wv": rng.standard_normal((D, NH, HD), dtype=np.float32) * scale,
        "wo": rng.standard_normal((D, NH, HD), dtype=np.float32) * scale,
    }
    out = kernel(**inputs)
    print(out.shape, out.dtype, float(np.abs(out).max()))
